# revision 28
# baseline (speedup 1.0000x reference)
"""MoE router kernel for Trainium2 (Bass/Tile), 8-core data-parallel SPMD.

Computes, per row b (B=4096 total, 512 per core):
  expert_logits[b,e] = sum_f x[b,e,f]*v_e[f] + ctx[b] + const      (E=64, f=7)
  gate_weights[b,:]  = softmax over top-8 of expert_logits (others 0)
  action_logits[b,a] = sum_e gate[b,e] * x_q[b,e,a]                 (a=3)
where v_e = We @ Ws[:H], v_c = Wc @ Ws[H:], ctx[b] = x_context[b] . v_c,
const = bc.Ws[H:] + be.Ws[:H] + bs.  (Associativity-folded: validated to
keep the top-8 sets identical to the reference on the fixed seed inputs.)
"""

import sys

for _p in ("/opt/trn_rl_repo", "/root/.axon_site/_ro/trn_rl_repo"):
    if _p not in sys.path:
        sys.path.append(_p)

import numpy as np

import concourse.bass as bass
import concourse.mybir as mybir
from concourse.bass_types import AP
from concourse.bass_utils import run_bass_kernel_spmd
from concourse.masks import make_identity
from concourse.tile import TileContext

F32 = mybir.dt.float32
ALU = mybir.AluOpType
ACTF = mybir.ActivationFunctionType

B, E, H, TOPK = 4096, 64, 512, 8
NCORES = 8
BS = B // NCORES          # rows per core = 512
P = 128                   # partitions
G = BS // P               # row groups per core = 4
FQ, FR, FK = 3, 2, 2      # per-expert feature widths
NF = FQ + FR + FK         # 7


def _bview(ap: AP, free_dims):
    """View `ap` with explicit free-dim (step, count) pairs (partition dim kept)."""
    return AP(ap.tensor, ap.offset, [list(ap.ap[0])] + [list(d) for d in free_dims])


def _split_waits(nc: bass.Bass) -> None:
    """walrus codegen in this environment supports a single sync-wait slot per
    compute instruction; peel extra waits onto cloned harmless same-engine ops
    placed just before. DMA/Drain (SP sequencer) instructions are left as-is."""
    import copy

    templates = nc._carrier_templates
    n = 0
    for f in nc.m.functions:
        for blk in f.blocks:
            out = []
            for ins in blk.instructions:
                si = ins.sync_info
                tmpl = templates.get(getattr(ins, "engine", None))
                if type(ins).__name__ == "InstISA" and tmpl is not None:
                    # walrus rejects the barrier's engine-nop ISA op; swap in a
                    # harmless same-engine memset with identical sync_info
                    car = copy.deepcopy(tmpl)
                    car.name = ins.name
                    car.sync_info = si
                    out.append(car)
                    continue
                is_drain = type(ins).__name__ == "InstDrain"
                if si is not None and si.on_wait and len(si.on_wait) > 1 and (tmpl is not None or is_drain):
                    waits = list(si.on_wait)
                    for w in waits[:-1]:
                        if is_drain:
                            car = mybir.InstDrain(name=f"waitcar-{n}", ins=[], outs=[])
                            car.engine = ins.engine
                        else:
                            car = copy.deepcopy(tmpl)
                            car.name = f"waitcar-{n}"
                        n += 1
                        car.sync_info = mybir.SyncInfo(on_wait=[w], on_update=[])
                        out.append(car)
                    si.on_wait = [waits[-1]]
                    ins.sync_info = si
                out.append(ins)
            blk.instructions = out


def _build_program() -> bass.Bass:
    nc = bass.Bass()

    # host-concatenated per-expert inputs: [q (192) | r (128) | k (128)]
    xin_d = nc.declare_dram_parameter("xin", [BS, E * NF], F32, isOutput=False)
    xc_d = nc.declare_dram_parameter("xc", [BS, 68], F32, isOutput=False)
    # host-stacked weights: rows = Wc(0:68) | bc(68) | We(69:76) | be(76)
    wx_d = nc.declare_dram_parameter("WX", [68 + 1 + NF + 1, H], F32, isOutput=False)
    ws_d = nc.declare_dram_parameter("Ws", [2 * H], F32, isOutput=False)
    bs_d = nc.declare_dram_parameter("bs", [1, 1], F32, isOutput=False)

    act_d = nc.declare_dram_parameter("act", [BS, FQ], F32, isOutput=True)
    gates_d = nc.declare_dram_parameter("gates", [BS, E], F32, isOutput=True)
    elog_d = nc.declare_dram_parameter("elog", [BS, E], F32, isOutput=True)

    NW = 68 + NF + 1 + 1  # stacked weight rows: Wc | We | bc | be = 77
    KC = H // P           # 4 chunks of the H contraction

    with TileContext(nc) as tc:
        with (
            tc.tile_pool(name="sb", bufs=1) as sb,
            tc.tile_pool(name="ps", bufs=1, space="PSUM") as ps,
        ):
            # ---------------- weight-folding preamble ----------------
            # NOTE: walrus lowers f32 matmul to an LDWEIGHTS struct that holds a
            # single sync-wait slot, so every PE instruction below must depend
            # on exactly one semaphore. All PE inputs are staged through DVE.
            # stacked X [77, 512]: rows = Wc(68) | bc | We(7) | be
            x_sb = sb.tile([NW, H], F32, tag="xw")
            nc.sync.dma_start(out=x_sb[:], in_=wx_d[:])
            x2_sb = sb.tile([NW, H], F32, tag="xw2")
            nc.vector.tensor_copy(out=x2_sb[:], in_=x_sb[:])
            # S [128, 2, KC]: S[p,w,c] = Ws[w*512 + c*128 + p]
            s_sb = sb.tile([P, 2, KC], F32, tag="s")
            nc.sync.dma_start(out=s_sb[:], in_=ws_d[:].rearrange("(w c p) -> p w c", p=P, c=KC, w=2))
            s2_sb = sb.tile([P, 2, KC], F32, tag="s2")
            nc.vector.tensor_copy(out=s2_sb[:], in_=s_sb[:])

            identg = sb.tile([P, P], F32, tag="identg")
            make_identity(nc, identg[:])
            ident = sb.tile([P, P], F32, tag="ident")
            nc.vector.tensor_copy(out=ident[:], in_=identg[:])

            # X^T chunks: [128, 4, 77]
            xt_sb = sb.tile([P, KC, NW], F32, tag="xt")
            for c in range(KC):
                pt = ps.tile([P, NW], F32, tag=f"pt{c}")
                nc.tensor.transpose(
                    out=pt[:], in_=x2_sb[:, c * P:(c + 1) * P], identity=ident[:NW, :NW]
                )
                nc.vector.tensor_copy(out=xt_sb[:, c, :], in_=pt[:])

            # row-vector folds on partition 0:
            #   psC [1, 69] = Ws_c^T @ (Wc | bc)^T  -> [v_c (68) | bc.Ws_c]
            #   psE [1, 8]  = Ws_e^T @ (We | be)^T  -> [v_e (7)  | be.Ws_e]
            psC = ps.tile([1, 69], F32, tag="psC")
            psE = ps.tile([1, 8], F32, tag="psE")
            for c in range(KC):
                nc.tensor.matmul(
                    out=psC[:], lhsT=s2_sb[:, 1, c:c + 1], rhs=xt_sb[:, c, 0:69],
                    start=(c == 0), stop=(c == KC - 1),
                )
            for c in range(KC):
                nc.tensor.matmul(
                    out=psE[:], lhsT=s2_sb[:, 0, c:c + 1], rhs=xt_sb[:, c, 69:NW],
                    start=(c == 0), stop=(c == KC - 1),
                )

            # broadcast source row [1, 76] = [v_c (68) | v_e (7) | const]
            bs_sb = sb.tile([1, 1], F32, tag="bs")
            nc.sync.dma_start(out=bs_sb[:], in_=bs_d[:])
            bsrc = sb.tile([1, 76], F32, tag="bsrc")
            nc.vector.tensor_copy(out=bsrc[0:1, 75:76], in_=bs_sb[:])
            nc.vector.tensor_copy(out=bsrc[0:1, 0:68], in_=psC[0:1, 0:68])
            nc.vector.tensor_copy(out=bsrc[0:1, 68:75], in_=psE[0:1, 0:7])
            nc.vector.tensor_tensor(out=bsrc[0:1, 75:76], in0=bsrc[0:1, 75:76], in1=psC[0:1, 68:69], op=ALU.add)
            nc.vector.tensor_tensor(out=bsrc[0:1, 75:76], in0=bsrc[0:1, 75:76], in1=psE[0:1, 7:8], op=ALU.add)

            ones1 = sb.tile([1, P], F32, tag="ones1")
            nc.vector.memset(ones1[:], 1.0)
            bc_ps = ps.tile([P, 76], F32, tag="bcps")
            nc.tensor.matmul(out=bc_ps[:], lhsT=ones1[:], rhs=bsrc[:], start=True, stop=True)
            bcast = sb.tile([P, 76], F32, tag="bcast")
            nc.scalar.copy(out=bcast[:], in_=bc_ps[:])
            vcB = bcast[:, 0:68]      # [128, 68] v_c replicated over partitions
            veB = bcast[:, 68:75]     # [128, 7]
            cB = bcast[:, 75:76]      # [128, 1] const

            # ---------------- main data-parallel body ----------------
            # inputs, grouped [128, G, cols]: row b = g*128 + p
            xin_sb = sb.tile([P, G, E * NF], F32, tag="xin")
            xc_sb = sb.tile([P, G, 68], F32, tag="xc")
            nc.sync.dma_start(out=xin_sb[:], in_=xin_d[:].rearrange("(g p) f -> p g f", p=P))
            nc.sync.dma_start(out=xc_sb[:], in_=xc_d[:].rearrange("(g p) f -> p g f", p=P))
            xq_sb = xin_sb[:, :, 0:E * FQ]
            xr_sb = xin_sb[:, :, E * FQ:E * (FQ + FR)]
            xk_sb = xin_sb[:, :, E * (FQ + FR):E * NF]

            # products arranged [128, G, E, 7], then reduce over f
            arr = sb.tile([P, G, E, NF], F32, tag="arr")
            nc.vector.tensor_tensor(
                out=arr[:, :, :, 0:FQ],
                in0=xq_sb.rearrange("p g (e f) -> p g e f", f=FQ),
                in1=_bview(veB[:, 0:FQ], [(0, G), (0, E), (1, FQ)]),
                op=ALU.mult,
            )
            nc.vector.tensor_tensor(
                out=arr[:, :, :, FQ:FQ + FR],
                in0=xr_sb.rearrange("p g (e f) -> p g e f", f=FR),
                in1=_bview(veB[:, FQ:FQ + FR], [(0, G), (0, E), (1, FR)]),
                op=ALU.mult,
            )
            nc.vector.tensor_tensor(
                out=arr[:, :, :, FQ + FR:NF],
                in0=xk_sb.rearrange("p g (e f) -> p g e f", f=FK),
                in1=_bview(veB[:, FQ + FR:NF], [(0, G), (0, E), (1, FK)]),
                op=ALU.mult,
            )
            elog_sb = sb.tile([P, G, E], F32, tag="elog")
            nc.vector.tensor_reduce(
                out=elog_sb[:], in_=arr[:], axis=mybir.AxisListType.X, op=ALU.add
            )

            # ctx[b] = x_ctx[b] . v_c + const
            ctx_sb = sb.tile([P, G], F32, tag="ctx")
            ctx_scr = sb.tile([P, G, 68], F32, tag="ctxscr")
            nc.vector.tensor_tensor(
                out=ctx_scr[:], in0=xc_sb[:],
                in1=_bview(vcB, [(0, G), (1, 68)]), op=ALU.mult,
            )
            nc.vector.tensor_reduce(
                out=ctx_sb[:], in_=ctx_scr[:], axis=mybir.AxisListType.X, op=ALU.add
            )
            nc.vector.tensor_scalar_add(ctx_sb[:], ctx_sb[:], cB)
            nc.vector.tensor_tensor(
                out=elog_sb[:], in0=elog_sb[:],
                in1=_bview(ctx_sb[:], [(1, G), (0, E)]), op=ALU.add
            )
            nc.sync.dma_start(out=elog_d[:].rearrange("(g p) e -> p g e", p=P), in_=elog_sb[:])

            # top-8 per row: m8 descending; M = m8[:,:,0], T = m8[:,:,7]
            m8 = sb.tile([P, G, 8], F32, tag="m8")
            for g in range(G):
                nc.vector.max(out=m8[:, g, :], in_=elog_sb[:, g, :])
            negM = sb.tile([P, G], F32, tag="negM")
            nc.vector.tensor_scalar_mul(negM[:], m8[:, :, 0], -1.0)

            ex_sb = sb.tile([P, G, E], F32, tag="ex")
            gu_sb = sb.tile([P, G, E], F32, tag="gu")
            z_sb = sb.tile([P, G], F32, tag="z")
            for g in range(G):
                nc.scalar.activation(
                    out=ex_sb[:, g, :], in_=elog_sb[:, g, :], func=ACTF.Exp,
                    bias=negM[:, g:g + 1], scale=1.0,
                )
            for g in range(G):
                # gu = (elog >= T) * exp(elog - M);  z = sum_e gu
                nc.vector.scalar_tensor_tensor(
                    out=gu_sb[:, g, :], in0=elog_sb[:, g, :], scalar=m8[:, g, 7:8],
                    in1=ex_sb[:, g, :], op0=ALU.is_ge, op1=ALU.mult,
                    accum_out=z_sb[:, g:g + 1],
                )
            zi_sb = sb.tile([P, G], F32, tag="zi")
            nc.vector.reciprocal(out=zi_sb[:], in_=z_sb[:])
            gates_sb = sb.tile([P, G, E], F32, tag="gates")
            for g in range(G):
                nc.scalar.mul(out=gates_sb[:, g, :], in_=gu_sb[:, g, :], mul=zi_sb[:, g:g + 1])
            nc.sync.dma_start(out=gates_d[:].rearrange("(g p) e -> p g e", p=P), in_=gates_sb[:])

            # action[b,a] = sum_e gates[b,e] * xq[b,e,a]
            atmp = sb.tile([P, G, FQ, E], F32, tag="atmp")
            nc.vector.tensor_tensor(
                out=atmp[:],
                in0=_bview(gates_sb[:], [(E, G), (0, FQ), (1, E)]),
                in1=_bview(xq_sb, [(E * NF, G), (1, FQ), (FQ, E)]),
                op=ALU.mult,
            )
            act_sb = sb.tile([P, G, FQ], F32, tag="act")
            nc.vector.tensor_reduce(
                out=act_sb[:], in_=atmp[:], axis=mybir.AxisListType.X, op=ALU.add
            )
            nc.sync.dma_start(out=act_d[:].rearrange("(g p) a -> p g a", p=P), in_=act_sb[:])

            # wait-carrier templates for _split_waits: harmless 1-elem ops,
            # one private scratch tile per engine (no cross-engine deps)
            wscr_v = sb.tile([1, 1], F32, tag="wscr_v")
            wscr_g = sb.tile([1, 1], F32, tag="wscr_g")
            wscr_a = sb.tile([1, 1], F32, tag="wscr_a")
            tmpl_v = nc.vector.memset(wscr_v[:], 0.0)
            tmpl_g = nc.gpsimd.memset(wscr_g[:], 0.0)
            tmpl_a = nc.scalar.mul(out=wscr_a[:], in_=wscr_a[:], mul=0.0)

    nc._carrier_templates = {
        mybir.EngineType.DVE: tmpl_v.ins,
        mybir.EngineType.Pool: tmpl_g.ins,
        mybir.EngineType.Activation: tmpl_a.ins,
    }
    return nc


_PROGRAM: bass.Bass | None = None


def _program() -> bass.Bass:
    """Program for hardware execution (waits split for walrus codegen)."""
    global _PROGRAM
    if _PROGRAM is None:
        _PROGRAM = _build_program()
        _split_waits(_PROGRAM)
    return _PROGRAM


def _in_maps(inputs: dict[str, np.ndarray]) -> list[dict[str, np.ndarray]]:
    f = lambda a: np.ascontiguousarray(np.asarray(a, dtype=np.float32))
    xin = np.concatenate([
        f(inputs["x_q_values"]).reshape(B, E * FQ),
        f(inputs["x_reward"]).reshape(B, E * FR),
        f(inputs["x_risk"]).reshape(B, E * FK),
    ], axis=1)
    xc = f(inputs["x_context"])
    shared = {
        "WX": np.ascontiguousarray(np.concatenate([
            f(inputs["Wc"]),
            f(inputs["bc"]).reshape(1, H),
            f(inputs["We"]),
            f(inputs["be"]).reshape(1, H),
        ], axis=0)),
        "Ws": f(inputs["Ws"]).reshape(2 * H),
        "bs": f(inputs["bs"]).reshape(1, 1),
    }
    maps = []
    for i in range(NCORES):
        sl = slice(i * BS, (i + 1) * BS)
        maps.append({
            "xin": np.ascontiguousarray(xin[sl]),
            "xc": np.ascontiguousarray(xc[sl]),
            **shared,
        })
    return maps


def kernel(**inputs: np.ndarray):
    nc = _program()
    res = run_bass_kernel_spmd(nc, _in_maps(inputs), list(range(NCORES))).results
    action = np.concatenate([res[i]["act"] for i in range(NCORES)], axis=0)
    gates = np.concatenate([res[i]["gates"] for i in range(NCORES)], axis=0)
    elog = np.concatenate([res[i]["elog"] for i in range(NCORES)], axis=0)
    return (action, gates, elog)
